# revision 16
# baseline (speedup 1.0000x reference)
"""Trainium2 Bass kernel for nn_DisentangleGraph (topk_masking).

Computes, for hidden (20000,256), H (20000,4096), int_emb (8,256):
  sim   = 10 * cosine(hidden, int_emb)                  (20000, 8)
  int_H = 2.0 where rank-within-column(sim) <= 6000     (top-6001 per column)
  H_out = concat([int_H, H], axis=1)                    (20000, 4104)
  degV  = rowsum(H_out);  degE = colmean of degV over nonzeros
  returns (H_out, degV**-0.5, degE**-0.5)

Sharding: node dimension split across 8 NeuronCores (2500 nodes each).
The per-column top-k threshold (the 6001-th largest sim value) is found with
a bisection over the value space after an AllGather of the (tiny) sim matrix;
degE needs an AllReduce of per-shard column sums.  Selection uses
`sim >= t` where t is the exact 6001-th largest value — equivalent to the
reference's double-argsort rank test whenever the threshold value is unique
in its column (holds for this input; verified against the reference).
"""

import numpy as np

import concourse.bacc as bacc
import concourse.mybir as mybir
from concourse import bass_utils
from concourse.tile import TileContext

F32 = mybir.dt.float32
ALU = mybir.AluOpType
ACTF = mybir.ActivationFunctionType
AX = mybir.AxisListType

N_NODES = 20000
NUM_EDGE = 4096
DIM = 256
K_FAC = 8
NC = 8
SHARD = N_NODES // NC          # 2500
NT = (SHARD + 127) // 128      # 20 tiles per shard
SEL_CNT = 6001.0               # rank <= floor(0.3*N) selects 6001 values
TEMP = 10.0
EPS = 1e-8
N_ITER = 36                    # bisection iterations (converges to exact fp32)
WIDTH = NUM_EDGE + K_FAC       # 4104
T_BUFS = 6

_CACHED = None


def _rows(i):
    return min(128, SHARD - i * 128)


def _build():
    nc = bacc.Bacc("TRN2", target_bir_lowering=False, debug=False, num_devices=NC)

    hid_d = nc.dram_tensor("hidden", [SHARD, DIM], F32, kind="ExternalInput")
    h_d = nc.dram_tensor("H", [SHARD, NUM_EDGE], F32, kind="ExternalInput")
    emb_d = nc.dram_tensor("int_emb", [K_FAC, DIM], F32, kind="ExternalInput")
    ident_d = nc.dram_tensor("ident", [128, 128], F32, kind="ExternalInput")
    g_d = nc.dram_tensor("G", [128, 128], F32, kind="ExternalInput")

    hout_d = nc.dram_tensor("Hout", [SHARD, WIDTH], F32, kind="ExternalOutput")
    degv_d = nc.dram_tensor("degV", [SHARD, 1], F32, kind="ExternalOutput")
    dege_d = nc.dram_tensor("degE", [WIDTH, 1], F32, kind="ExternalOutput")

    with TileContext(nc) as tc:
        with (
            tc.tile_pool(name="const", bufs=1) as cpool,
            tc.tile_pool(name="hid", bufs=3) as hpool,
            tc.tile_pool(name="hnt", bufs=3) as tpool_hnt,
            tc.tile_pool(name="bigT", bufs=T_BUFS) as Tpool,
            tc.tile_pool(name="small", bufs=3) as spool,
            tc.tile_pool(name="psA", bufs=1, space="PSUM") as psA,
            tc.tile_pool(name="psB", bufs=2, space="PSUM") as psB,
            tc.tile_pool(name="psC", bufs=2, space="PSUM") as psC,
            tc.tile_pool(name="dram", bufs=1, space="DRAM") as dpool,
        ):
            # ---------------- constants ----------------
            ident = cpool.tile([128, 128], F32)
            nc.sync.dma_start(ident[:], ident_d[:])
            gmat = cpool.tile([128, 128], F32)
            nc.sync.dma_start(gmat[:], g_d[:])
            ones_row = cpool.tile([1, 128], F32)
            nc.vector.memset(ones_row[:], 1.0)

            # ---------------- normalized int_emb, transposed ----------------
            emb = cpool.tile([K_FAC, DIM], F32)
            nc.sync.dma_start(emb[:], emb_d[:])
            esq = cpool.tile([K_FAC, DIM], F32)
            ess = cpool.tile([K_FAC, 1], F32)
            nc.scalar.activation(esq[:], emb[:], ACTF.Square, accum_out=ess[:])
            enorm = cpool.tile([K_FAC, 1], F32)
            nc.scalar.sqrt(enorm[:], ess[:])
            nc.vector.tensor_scalar_max(enorm[:], enorm[:], EPS)
            erin = cpool.tile([K_FAC, 1], F32)
            nc.vector.reciprocal(erin[:], enorm[:])
            en = cpool.tile([K_FAC, DIM], F32)
            nc.vector.tensor_scalar_mul(en[:], emb[:], erin[:])
            enT = cpool.tile([128, 2 * K_FAC], F32)  # chunk c at [:, c*8:(c+1)*8]
            for c in range(2):
                pt = psB.tile([128, 128], F32, tag="tr")
                nc.tensor.transpose(
                    pt[:128, :K_FAC], en[:, c * 128 : (c + 1) * 128], ident[:K_FAC, :K_FAC]
                )
                nc.scalar.copy(enT[:, c * K_FAC : (c + 1) * K_FAC], pt[:128, :K_FAC])

            # ---------------- phase A: local sim + transposed sim ----------------
            sim_loc = cpool.tile([128, NT, K_FAC], F32)   # node-major local sim
            simT_loc = cpool.tile([K_FAC, SHARD], F32)    # column-major local sim
            for i in range(NT):
                rows = _rows(i)
                r0 = i * 128
                ht = hpool.tile([128, DIM], F32, tag="h")
                nc.sync.dma_start(ht[:rows, :], hid_d[r0 : r0 + rows, :])
                sq = hpool.tile([128, DIM], F32, tag="sq")
                ss = spool.tile([128, 1], F32, tag="ss")
                nc.scalar.activation(sq[:rows, :], ht[:rows, :], ACTF.Square,
                                     accum_out=ss[:rows, :])
                nrm = spool.tile([128, 1], F32, tag="nrm")
                nc.scalar.sqrt(nrm[:rows, :], ss[:rows, :])
                nc.vector.tensor_scalar_max(nrm[:rows, :], nrm[:rows, :], EPS)
                rin = spool.tile([128, 1], F32, tag="rin")
                nc.vector.reciprocal(rin[:rows, :], nrm[:rows, :])
                hn = hpool.tile([128, DIM], F32, tag="hn")
                nc.vector.tensor_scalar_mul(hn[:rows, :], ht[:rows, :], rin[:rows, :])

                hnT = tpool_hnt.tile([128, 256], F32, tag="hnT")
                for c in range(2):
                    pt = psB.tile([128, 128], F32, tag="tr")
                    nc.tensor.transpose(
                        pt[:128, :rows],
                        hn[:rows, c * 128 : (c + 1) * 128],
                        ident[:rows, :rows],
                    )
                    nc.vector.tensor_copy(hnT[:, c * 128 : c * 128 + rows],
                                          pt[:128, :rows])
                psim = psC.tile([128, 128], F32, tag="mm")
                for c in range(2):
                    nc.tensor.matmul(
                        psim[:rows, :K_FAC],
                        hnT[:, c * 128 : c * 128 + rows],
                        enT[:, c * K_FAC : (c + 1) * K_FAC],
                        start=(c == 0),
                        stop=(c == 1),
                    )
                # sim = TEMP * (hn @ en.T)
                nc.scalar.mul(sim_loc[:rows, i, :], psim[:rows, :K_FAC], TEMP)
                ptT = psC.tile([128, 128], F32, tag="mm")
                nc.tensor.transpose(
                    ptT[:K_FAC, :rows], sim_loc[:rows, i, :], ident[:rows, :rows]
                )
                nc.scalar.copy(simT_loc[:, r0 : r0 + rows], ptT[:K_FAC, :rows])

            # ---------------- AllGather sim ----------------
            simT_d = dpool.tile([K_FAC, SHARD], F32)
            nc.gpsimd.dma_start(simT_d[:], simT_loc[:])
            simfull_d = dpool.tile([NC * K_FAC, SHARD], F32, addr_space="Shared")
            nc.gpsimd.collective_compute(
                "AllGather",
                ALU.bypass,
                replica_groups=[list(range(NC))],
                ins=[simT_d[:].opt()],
                outs=[simfull_d[:].opt()],
            )
            # grouped layout: partition p = (q, h), q = rank*8+f, h in {0,1}
            # -> column of partition p is (p//2) % 8; 16 partitions per column.
            sim_g = cpool.tile([128, SHARD // 2], F32)
            nc.sync.dma_start(
                sim_g[:], simfull_d[:].rearrange("q (h i) -> (q h) i", h=2)
            )

            # ---------------- bisection for per-column threshold ----------------
            lo = cpool.tile([128, 1], F32)
            hi = cpool.tile([128, 1], F32)
            mid = cpool.tile([128, 1], F32)
            nc.vector.memset(lo[:], -12.0)
            nc.vector.memset(hi[:], 12.0)
            nc.vector.memset(mid[:], 0.0)
            cmp_buf = cpool.tile([128, SHARD // 2], F32)
            for it in range(N_ITER):
                pcnt = spool.tile([128, 1], F32, tag="pcnt")
                nc.vector.tensor_scalar(
                    out=cmp_buf[:],
                    in0=sim_g[:],
                    scalar1=mid[:],
                    scalar2=None,
                    op0=ALU.is_ge,
                    op1=ALU.add,
                    accum_out=pcnt[:],
                )
                pc = psC.tile([128, 128], F32, tag="mm")
                nc.tensor.matmul(pc[:128, :1], gmat[:], pcnt[:], start=True, stop=True)
                cnt = spool.tile([128, 1], F32, tag="cnt")
                nc.scalar.copy(cnt[:], pc[:128, :1])
                geq = spool.tile([128, 1], mybir.dt.uint32, tag="geq")
                nc.vector.tensor_scalar(
                    out=geq[:], in0=cnt[:], scalar1=SEL_CNT, scalar2=None,
                    op0=ALU.is_ge,
                )
                ltq = spool.tile([128, 1], mybir.dt.uint32, tag="ltq")
                nc.vector.tensor_scalar(
                    out=ltq[:], in0=cnt[:], scalar1=SEL_CNT, scalar2=None,
                    op0=ALU.is_lt,
                )
                nc.vector.copy_predicated(lo[:], geq[:], mid[:])
                nc.vector.copy_predicated(hi[:], ltq[:], mid[:])
                sm = spool.tile([128, 1], F32, tag="sm")
                nc.vector.tensor_tensor(sm[:], lo[:], hi[:], ALU.add)
                nc.scalar.mul(mid[:], sm[:], 0.5)

            # threshold row (1, 8): column f lives (a.o.) on partition 2f
            th_row = cpool.tile([1, K_FAC], F32)
            nc.sync.dma_start(th_row[:], lo[0:16:2, :])
            pbc = psC.tile([128, 128], F32, tag="mm")
            nc.tensor.matmul(pbc[:128, :K_FAC], ones_row[:], th_row[:],
                             start=True, stop=True)
            thr = cpool.tile([128, K_FAC], F32)
            nc.scalar.copy(thr[:], pbc[:128, :K_FAC])

            # ---------------- phase B: stream H, build H_out, accumulate sums ----
            # Column-sum accumulators: chunk c in {0..8} (8 H chunks of 512
            # cols + the 8 int cols) accumulates [colsum(mask), colsum(
            # mask*degV)] as a (2, 512) PSUM region.  PE matmul outputs must
            # start at partition 0/32/64, so chunk c lives in PSUM bank
            # paccs[c // 3] at partition offset 32 * (c % 3).
            paccs = [
                psA.tile([128, 512], F32, tag=f"pacc{t}", name=f"pacc{t}")
                for t in range(3)
            ]

            def acc_slice(c, width=512):
                return paccs[c // 3][32 * (c % 3) : 32 * (c % 3) + 2, :width]
            degv_all = cpool.tile([128, NT], F32)
            for i in range(NT):
                rows = _rows(i)
                r0 = i * 128
                T = Tpool.tile([128, WIDTH], F32, tag="T")
                nc.sync.dma_start(T[:rows, K_FAC:], h_d[r0 : r0 + rows, :])
                r = spool.tile([128, 1], F32, tag="r")
                nc.vector.reduce_sum(r[:rows, :], T[:rows, K_FAC:], axis=AX.X)
                sel = spool.tile([128, K_FAC], F32, tag="sel")
                nc.vector.tensor_tensor(
                    sel[:rows, :], sim_loc[:rows, i, :], thr[:rows, :], ALU.is_ge
                )
                s2 = spool.tile([128, 1], F32, tag="s2")
                nc.scalar.activation(
                    T[:rows, 0:K_FAC], sel[:rows, :], ACTF.Copy, scale=2.0,
                    accum_out=s2[:rows, :],
                )
                dv = spool.tile([128, 1], F32, tag="dv")
                nc.vector.tensor_tensor(dv[:rows, :], r[:rows, :], s2[:rows, :],
                                        ALU.add)
                lw = spool.tile([128, 2], F32, tag="lw")
                nc.vector.memset(lw[:rows, 0:1], 1.0)
                nc.vector.tensor_copy(lw[:rows, 1:2], dv[:rows, :])
                for c in range(8):
                    nc.tensor.matmul(
                        acc_slice(c),
                        lw[:rows, :],
                        T[:rows, K_FAC + 512 * c : K_FAC + 512 * (c + 1)],
                        start=(i == 0),
                        stop=(i == NT - 1),
                    )
                nc.tensor.matmul(
                    acc_slice(8, K_FAC),
                    lw[:rows, :],
                    T[:rows, 0:K_FAC],
                    start=(i == 0),
                    stop=(i == NT - 1),
                )
                rec = spool.tile([128, 1], F32, tag="rec")
                nc.vector.reciprocal(rec[:rows, :], dv[:rows, :])
                nc.scalar.sqrt(degv_all[:rows, i : i + 1], rec[:rows, :])
                nc.sync.dma_start(hout_d[r0 : r0 + rows, :], T[:rows, :])

            # degV out: degv_all[p, i] -> degV[i*128 + p]
            full = (NT - 1) * 128
            nc.sync.dma_start(
                degv_d[:full, :].rearrange("(i p) one -> p (i one)", p=128),
                degv_all[:, : NT - 1],
            )
            nc.sync.dma_start(
                degv_d[full:, :], degv_all[: SHARD - full, NT - 1 : NT]
            )

            # ---------------- phase C: AllReduce partials, compute degE --------
            # Compute-engine partition bases must be multiples of 32, so pack
            # chunk c's colsum(mask) row at partition c and its
            # colsum(mask*degV) row at partition 32+c via SBUF-SBUF DMAs.
            acc_sb = cpool.tile([18, 512], F32)
            nc.vector.memset(acc_sb[0:18, :], 0.0)
            mirrors = [
                cpool.tile([66, 512], F32, name=f"mir{t}") for t in range(3)
            ]
            for c in range(9):
                w = 512 if c < 8 else K_FAC
                off = 32 * (c % 3)
                nc.vector.tensor_copy(
                    mirrors[c // 3][off : off + 2, :w], acc_slice(c, w)
                )
                # cnt row -> partition c, wsum row -> partition 9 + c
                nc.sync.dma_start(
                    acc_sb[c : c + 1, :w], mirrors[c // 3][off : off + 1, :w]
                )
                nc.sync.dma_start(
                    acc_sb[9 + c : 10 + c, :w],
                    mirrors[c // 3][off + 1 : off + 2, :w],
                )
            ar_in = dpool.tile([18, 512], F32)
            nc.gpsimd.dma_start(ar_in[:], acc_sb[0:18, :])
            ar_out = dpool.tile([18, 512], F32, addr_space="Shared")
            nc.gpsimd.collective_compute(
                "AllReduce",
                ALU.add,
                replica_groups=[list(range(NC))],
                ins=[ar_in[:].opt()],
                outs=[ar_out[:].opt()],
            )
            # rows 0..8 = cnt (int chunk carries 2*cnt, 2*wsum; factors
            # cancel in the ratio), rows 9..17 = wsum.
            cnt_t = cpool.tile([9, 512], F32)
            nc.gpsimd.dma_start(cnt_t[0:9, :], ar_out[0:9, :])
            wsum_t = cpool.tile([9, 512], F32)
            nc.gpsimd.dma_start(wsum_t[0:9, :], ar_out[9:18, :])

            cm = cpool.tile([9, 512], F32)
            nc.vector.tensor_scalar_max(cm[0:9, :], cnt_t[0:9, :], 1.0)
            # clamp: only affects the never-output padding cells of the int
            # row (wsum there is 0, and sqrt(inf) faults the scalar engine)
            nc.vector.tensor_scalar_max(wsum_t[0:9, :], wsum_t[0:9, :], 1e-30)
            rw = cpool.tile([9, 512], F32)
            nc.vector.reciprocal(rw[0:9, :], wsum_t[0:9, :])
            rr = cpool.tile([9, 512], F32)
            nc.vector.tensor_tensor(rr[0:9, :], cm[0:9, :], rw[0:9, :], ALU.mult)
            dege_all = cpool.tile([9, 512], F32)
            nc.scalar.sqrt(dege_all[0:9, :], rr[0:9, :])
            nc.sync.dma_start(
                dege_d[K_FAC:, :].rearrange("(c k) one -> c (k one)", c=8),
                dege_all[0:8, :],
            )
            nc.sync.dma_start(
                dege_d[0:K_FAC, :].rearrange("k one -> one k"),
                dege_all[8:9, 0:K_FAC],
            )

    nc.finalize()
    return nc


def _constants():
    ident = np.eye(128, dtype=np.float32)
    p = np.arange(128)
    gmat = ((p[:, None] // 2) % 8 == (p[None, :] // 2) % 8).astype(np.float32)
    return ident, gmat


def kernel(hidden, H, int_emb):
    global _CACHED
    if _CACHED is None:
        _CACHED = _build()
    nc = _CACHED
    hidden = np.ascontiguousarray(hidden, dtype=np.float32)
    H = np.ascontiguousarray(H, dtype=np.float32)
    int_emb = np.ascontiguousarray(int_emb, dtype=np.float32)
    ident, gmat = _constants()
    in_maps = [
        {
            "hidden": hidden[r * SHARD : (r + 1) * SHARD],
            "H": H[r * SHARD : (r + 1) * SHARD],
            "int_emb": int_emb,
            "ident": ident,
            "G": gmat,
        }
        for r in range(NC)
    ]
    res = bass_utils.run_bass_kernel_spmd(
        nc, in_maps, core_ids=list(range(NC)), trace=False
    )
    outs = res.results
    H_out = np.concatenate([outs[r]["Hout"] for r in range(NC)], axis=0)
    degV = np.concatenate([outs[r]["degV"] for r in range(NC)], axis=0)
    degE = outs[0]["degE"]
    return H_out, degV, degE
